# revision 8
# baseline (speedup 1.0000x reference)
"""Chamfer distance kernel for Trainium2 (Bass/Tile), 8-core SPMD.

Problem: x [16, 4096, 3], y [16, 4096, 3] fp32.
  d[b,n,m] = ||x[b,n] - y[b,m]||^2
  out = mean_n(min_m d) + mean_m(min_n d)   (scalar fp32)

Strategy:
  - Data-parallel over batch: 2 batches per core.
  - d = x2 + y2 - 2*x.y computed on TensorE as one K=13 matmul using an
    fp16 hi/lo split of the fp32 inputs (error ~1e-5, exact enough).
    4-way PE row-tiling (tile_position) since K=13 <= 32.
  - ScalarE converts each PSUM chunk to fp16 in SBUF (1x rate).
  - VectorE (2x fp16 mode):
      min_l: tensor_tensor_reduce folds the chunk pairwise and row-min
             reduces it in a single op.
      min_r: running elementwise-min buffer rm[128, M] across x-tiles.
  - Final 128-partition min of rm and all means are done on the host
    (tiny: a few MB of fp16 partials).
"""

import os
import numpy as np

_TRNREPO = "/opt/trn_rl_repo"
try:
    import concourse.bass as bass
except ImportError:  # pragma: no cover
    import sys

    sys.path.insert(0, _TRNREPO)
    import concourse.bass as bass

from contextlib import ExitStack

import concourse.bacc as bacc
import concourse.tile as tile
from concourse import mybir
from concourse.bass_utils import run_bass_kernel_spmd

F16 = mybir.dt.float16
F32 = mybir.dt.float32

B, N, M, D = 16, 4096, 4096, 3
NCORES = 8
BPC = B // NCORES  # batches per core

KP = 16  # stationary partition rows per PE band (13 used, 3 zero)

# knobs for the full-size build
TRACE = False
LAST = {}


def _dims(n, m):
    nt = n // 128          # x tiles
    mq = m // 4            # y columns per PE band (quarter)
    ch = min(512, mq)      # columns per matmul (one psum bank at fp32)
    nh = mq // ch          # chunks per x-tile
    return nt, mq, ch, nh


def build_program(b_pc=BPC, n=N, m=M):
    """Emit the per-core Tile program. Returns the Bass object."""
    nt, mq, ch, nh = _dims(n, m)
    nc = bacc.Bacc("TRN2", target_bir_lowering=False)

    xs_d = nc.declare_dram_parameter("xs", [128, b_pc, n], F16, isOutput=False)
    ys_d = nc.declare_dram_parameter(
        "ys", [128, b_pc, nh, 4, ch], F16, isOutput=False
    )
    ml_d = nc.declare_dram_parameter(
        "ml_out", [b_pc, 128, nt * nh], F16, isOutput=True
    )
    rm_d = nc.declare_dram_parameter(
        "rm_out", [b_pc, 128, 4, mq], F16, isOutput=True
    )

    with ExitStack() as ctx:
        tc = ctx.enter_context(tile.TileContext(nc))
        xs_pool = ctx.enter_context(tc.tile_pool(name="xs", bufs=1))
        ys_pool = ctx.enter_context(tc.tile_pool(name="ys", bufs=1))
        psum_pool = ctx.enter_context(tc.tile_pool(name="psum", bufs=2, space="PSUM"))
        c16_pool = ctx.enter_context(tc.tile_pool(name="c16", bufs=3))
        fold_pool = ctx.enter_context(tc.tile_pool(name="fold", bufs=2))
        rm_pool = ctx.enter_context(tc.tile_pool(name="rm", bufs=2))
        ml_pool = ctx.enter_context(tc.tile_pool(name="ml", bufs=2))

        xs = xs_pool.tile([128, b_pc, n], F16)
        ys = ys_pool.tile([128, b_pc, nh, 4, ch], F16)

        # stage inputs; split into pieces so compute can start early
        for b in range(b_pc):
            npiece = max(1, n // 1024)
            step = n // npiece
            for i in range(npiece):
                nc.sync.dma_start(
                    xs[:, b, i * step:(i + 1) * step],
                    xs_d[:, b, i * step:(i + 1) * step],
                )
            for h in range(nh):
                nc.sync.dma_start(ys[:, b, h], ys_d[:, b, h])

        for b in range(b_pc):
            rm = rm_pool.tile([128, 4, mq], F16)
            ml = ml_pool.tile([128, nt * nh], F16)
            for t in range(nt):
                for h in range(nh):
                    pch = psum_pool.tile([128, 4, ch], F32)
                    for r in range(4):
                        nc.tensor.matmul(
                            pch[:, r, :],
                            xs[32 * r:32 * r + KP, b, 128 * t:128 * (t + 1)],
                            ys[32 * r:32 * r + KP, b, h, r, :],
                            start=True,
                            stop=True,
                            tile_position=(32 * r, 0),
                        )
                    c16 = c16_pool.tile([128, 4, ch], F16)
                    nc.scalar.copy(c16[:, :, :], pch[:, :, :])
                    # min_l partial for this chunk: pairwise-min tree (fp16
                    # tensor_tensor runs in the 2x DVE mode; tensor_reduce is
                    # 1x, so fold down to ch//2 before the final reduce)
                    fold = fold_pool.tile([128, 2, ch], F16)
                    nc.vector.tensor_tensor(
                        fold[:, :, :], c16[:, 0:2, :], c16[:, 2:4, :],
                        mybir.AluOpType.min,
                    )
                    f2 = fold_pool.tile([128, ch], F16)
                    nc.vector.tensor_tensor(
                        f2[:, :], fold[:, 0, :], fold[:, 1, :],
                        mybir.AluOpType.min,
                    )
                    f3 = fold_pool.tile([128, ch // 2], F16)
                    nc.vector.tensor_tensor(
                        f3[:, :], f2[:, 0:ch // 2], f2[:, ch // 2:ch],
                        mybir.AluOpType.min,
                    )
                    nc.vector.tensor_reduce(
                        ml[:, t * nh + h : t * nh + h + 1], f3[:, :],
                        mybir.AxisListType.X, mybir.AluOpType.min,
                    )
                    # min_r running elementwise min across x-tiles
                    rms = rm[:, :, h * ch:(h + 1) * ch]
                    if t == 0:
                        nc.vector.tensor_copy(rms, c16[:, :, :])
                    else:
                        nc.vector.tensor_tensor(
                            rms, c16[:, :, :], rms, mybir.AluOpType.min
                        )
            nc.sync.dma_start(ml_d[b], ml[:, :])
            nc.sync.dma_start(rm_d[b], rm[:, :, :])
    nc.compile()
    return nc


def _split16(a):
    """fp32 array -> (hi, lo) fp16 arrays with hi+lo ~= a."""
    hi = a.astype(np.float16)
    lo = (a - hi.astype(np.float32)).astype(np.float16)
    return hi, lo


def prep_inputs(x, y, b_pc=BPC, n=N, m=M):
    """Build per-core augmented fp16 operands.

    Returns list of in_maps (one per core)."""
    nt, mq, ch, nh = _dims(n, m)
    x = np.asarray(x, dtype=np.float32)
    y = np.asarray(y, dtype=np.float32)
    nb = x.shape[0]

    a = -2.0 * x                                # [B, n, 3]
    ah, al = _split16(a)
    yh, yl = _split16(y)
    x2 = np.sum(x.astype(np.float64) ** 2, axis=-1).astype(np.float32)
    y2 = np.sum(y.astype(np.float64) ** 2, axis=-1).astype(np.float32)
    x2h, x2l = _split16(x2)
    y2h, y2l = _split16(y2)
    ones_x = np.ones_like(x2h)
    ones_y = np.ones_like(y2h)

    # K' = 13 rows
    S = np.stack(
        [ah[..., 0], ah[..., 1], ah[..., 2],
         ah[..., 0], ah[..., 1], ah[..., 2],
         al[..., 0], al[..., 1], al[..., 2],
         x2h, x2l, ones_x, ones_x],
        axis=1,
    )  # [B, 13, n]
    V = np.stack(
        [yh[..., 0], yh[..., 1], yh[..., 2],
         yl[..., 0], yl[..., 1], yl[..., 2],
         yh[..., 0], yh[..., 1], yh[..., 2],
         ones_y, ones_y, y2h, y2l],
        axis=1,
    )  # [B, 13, m]

    in_maps = []
    for c in range(nb // b_pc):
        xs = np.zeros((128, b_pc, n), dtype=np.float16)
        ys = np.zeros((128, b_pc, nh, 4, ch), dtype=np.float16)
        for b in range(b_pc):
            gb = c * b_pc + b
            for r in range(4):
                xs[32 * r:32 * r + 13, b, :] = S[gb]
                # V for band r: y columns [r*mq + h*ch + j]
                vq = V[gb][:, r * mq:(r + 1) * mq].reshape(13, nh, ch)
                ys[32 * r:32 * r + 13, b, :, r, :] = vq
        in_maps.append({"xs": xs, "ys": ys})
    return in_maps


def finish(results, b_pc=BPC, n=N, m=M):
    """Combine per-core partial outputs into the scalar loss."""
    tot_l = 0.0
    tot_r = 0.0
    nb = 0
    for res in results:
        ml = np.asarray(res["ml_out"], dtype=np.float64)   # [b_pc, 128, nt*nh]
        rm = np.asarray(res["rm_out"], dtype=np.float64)   # [b_pc, 128, 4, mq]
        nt, mq, ch, nh = _dims(n, m)
        # ml columns t*nh+h hold min over that chunk; min over h per tile
        mlv = ml.reshape(b_pc, 128, nt, nh).min(axis=3)    # [b_pc, 128, nt]
        tot_l += mlv.sum()
        tot_r += rm.min(axis=1).sum()                      # min over partitions
        nb += b_pc
    loss = tot_l / (nb * n) + tot_r / (nb * m)
    return np.float32(loss)


_BUILT = {}


def kernel(x, y):
    x = np.asarray(x)
    y = np.asarray(y)
    assert x.shape == (B, N, D) and y.shape == (B, M, D), (x.shape, y.shape)

    if "nc" not in _BUILT:
        _BUILT["nc"] = build_program()
    nc = _BUILT["nc"]

    in_maps = prep_inputs(x, y)
    core_ids = list(range(NCORES))
    res = run_bass_kernel_spmd(nc, in_maps, core_ids, trace=TRACE)
    LAST["results"] = res
    return finish(res.results)


if __name__ == "__main__":
    xs = np.random.RandomState(0).randn(B, N, D).astype(np.float32)
    ys = np.random.RandomState(1).randn(B, M, D).astype(np.float32)
    print(kernel(xs, ys))


# revision 13
# speedup vs baseline: 1.2160x; 1.2160x over previous
"""Chamfer distance kernel for Trainium2 (Bass/Tile), 8-core SPMD.

Problem: x [16, 4096, 3], y [16, 4096, 3] fp32.
  d[b,n,m] = ||x[b,n] - y[b,m]||^2
  out = mean_n(min_m d) + mean_m(min_n d)   (scalar fp32)

Strategy:
  - Data-parallel over batch: 2 batches per core.
  - d = x2 + y2 - 2*x.y computed on TensorE as one K=13 matmul using an
    fp16 hi/lo split of the fp32 inputs (error ~1e-5, exact enough).
    4-way PE row-tiling (tile_position) since K=13 <= 32.
  - ScalarE converts each PSUM chunk to fp16 in SBUF (1x rate).
  - VectorE (2x fp16 mode):
      min_l: tensor_tensor_reduce folds the chunk pairwise and row-min
             reduces it in a single op.
      min_r: running elementwise-min buffer rm[128, M] across x-tiles.
  - Final 128-partition min of rm and all means are done on the host
    (tiny: a few MB of fp16 partials).
"""

import os
import numpy as np

_TRNREPO = "/opt/trn_rl_repo"
try:
    import concourse.bass as bass
except ImportError:  # pragma: no cover
    import sys

    sys.path.insert(0, _TRNREPO)
    import concourse.bass as bass

from contextlib import ExitStack

import concourse.bacc as bacc
import concourse.tile as tile
from concourse import mybir
from concourse.bass_utils import run_bass_kernel_spmd

F16 = mybir.dt.float16
F32 = mybir.dt.float32

B, N, M, D = 16, 4096, 4096, 3
NCORES = 8
BPC = B // NCORES  # batches per core

KP = 16  # stationary partition rows per PE band (13 used, 3 zero)

# knobs for the full-size build
TRACE = False
LAST = {}


def _dims(n, m):
    nt = n // 128          # x tiles
    mq = m // 4            # y columns per PE band (quarter)
    ch = min(512, mq)      # columns per matmul (one psum bank at fp32)
    nh = mq // ch          # chunks per x-tile
    return nt, mq, ch, nh


def build_program(b_pc=BPC, n=N, m=M):
    """Emit the per-core Tile program. Returns the Bass object."""
    nt, mq, ch, nh = _dims(n, m)
    nc = bacc.Bacc("TRN2", target_bir_lowering=False)

    xs_d = nc.declare_dram_parameter("xs", [128, b_pc, n], F16, isOutput=False)
    ys_d = nc.declare_dram_parameter(
        "ys", [128, b_pc, nh, 4, ch], F16, isOutput=False
    )
    ml_d = nc.declare_dram_parameter(
        "ml_out", [b_pc, nt * nh, 128, ch], F16, isOutput=True
    )
    rm_d = nc.declare_dram_parameter(
        "rm_out", [b_pc, 128, 4, mq], F16, isOutput=True
    )

    with ExitStack() as ctx:
        tc = ctx.enter_context(tile.TileContext(nc))
        xs_pool = ctx.enter_context(tc.tile_pool(name="xs", bufs=1))
        ys_pool = ctx.enter_context(tc.tile_pool(name="ys", bufs=1))
        psum_pool = ctx.enter_context(tc.tile_pool(name="psum", bufs=2, space="PSUM"))
        c16_pool = ctx.enter_context(tc.tile_pool(name="c16", bufs=3))
        fold_pool = ctx.enter_context(tc.tile_pool(name="fold", bufs=2))
        rm_pool = ctx.enter_context(tc.tile_pool(name="rm", bufs=2))
        ml_pool = ctx.enter_context(tc.tile_pool(name="ml", bufs=2))

        xs = xs_pool.tile([128, b_pc, n], F16)
        ys = ys_pool.tile([128, b_pc, nh, 4, ch], F16)

        # stage inputs; split into pieces so compute can start early
        for b in range(b_pc):
            npiece = max(1, n // 1024)
            step = n // npiece
            for i in range(npiece):
                nc.sync.dma_start(
                    xs[:, b, i * step:(i + 1) * step],
                    xs_d[:, b, i * step:(i + 1) * step],
                )
            for h in range(nh):
                nc.sync.dma_start(ys[:, b, h], ys_d[:, b, h])

        for b in range(b_pc):
            rm = rm_pool.tile([128, 4, mq], F16)
            for t in range(nt):
                for h in range(nh):
                    pch = psum_pool.tile([128, 4, ch], F32)
                    for r in range(4):
                        nc.tensor.matmul(
                            pch[:, r, :],
                            xs[32 * r:32 * r + KP, b, 128 * t:128 * (t + 1)],
                            ys[32 * r:32 * r + KP, b, h, r, :],
                            start=True,
                            stop=True,
                            tile_position=(32 * r, 0),
                        )
                    c16 = c16_pool.tile([128, 4, ch], F16)
                    nc.scalar.copy(c16[:, :, :], pch[:, :, :])
                    # min_l partial for this chunk: pairwise-min tree (fp16
                    # tensor_tensor runs in the 2x DVE mode; tensor_reduce is
                    # 1x, so fold down to ch//2 before the final reduce)
                    fold = fold_pool.tile([128, 2, ch], F16)
                    nc.vector.tensor_tensor(
                        fold[:, :, :], c16[:, 0:2, :], c16[:, 2:4, :],
                        mybir.AluOpType.min,
                    )
                    f2 = fold_pool.tile([128, ch], F16)
                    nc.vector.tensor_tensor(
                        f2[:, :], fold[:, 0, :], fold[:, 1, :],
                        mybir.AluOpType.min,
                    )
                    # host finishes the last min over ch columns
                    nc.sync.dma_start(ml_d[b, t * nh + h], f2[:, :])
                    # min_r running elementwise min across x-tiles
                    rms = rm[:, :, h * ch:(h + 1) * ch]
                    if t == 0:
                        nc.vector.tensor_copy(rms, c16[:, :, :])
                    else:
                        nc.vector.tensor_tensor(
                            rms, c16[:, :, :], rms, mybir.AluOpType.min
                        )
            nc.sync.dma_start(rm_d[b], rm[:, :, :])
    nc.compile()
    return nc


def _split16(a):
    """fp32 array -> (hi, lo) fp16 arrays with hi+lo ~= a."""
    hi = a.astype(np.float16)
    lo = (a - hi.astype(np.float32)).astype(np.float16)
    return hi, lo


def prep_inputs(x, y, b_pc=BPC, n=N, m=M):
    """Build per-core augmented fp16 operands.

    Returns list of in_maps (one per core)."""
    nt, mq, ch, nh = _dims(n, m)
    x = np.asarray(x, dtype=np.float32)
    y = np.asarray(y, dtype=np.float32)
    nb = x.shape[0]

    a = -2.0 * x                                # [B, n, 3]
    ah, al = _split16(a)
    yh, yl = _split16(y)
    x2 = np.sum(x.astype(np.float64) ** 2, axis=-1).astype(np.float32)
    y2 = np.sum(y.astype(np.float64) ** 2, axis=-1).astype(np.float32)
    x2h, x2l = _split16(x2)
    y2h, y2l = _split16(y2)
    ones_x = np.ones_like(x2h)
    ones_y = np.ones_like(y2h)

    # K' = 13 rows
    S = np.stack(
        [ah[..., 0], ah[..., 1], ah[..., 2],
         ah[..., 0], ah[..., 1], ah[..., 2],
         al[..., 0], al[..., 1], al[..., 2],
         x2h, x2l, ones_x, ones_x],
        axis=1,
    )  # [B, 13, n]
    V = np.stack(
        [yh[..., 0], yh[..., 1], yh[..., 2],
         yl[..., 0], yl[..., 1], yl[..., 2],
         yh[..., 0], yh[..., 1], yh[..., 2],
         ones_y, ones_y, y2h, y2l],
        axis=1,
    )  # [B, 13, m]

    in_maps = []
    for c in range(nb // b_pc):
        xs = np.zeros((128, b_pc, n), dtype=np.float16)
        ys = np.zeros((128, b_pc, nh, 4, ch), dtype=np.float16)
        for b in range(b_pc):
            gb = c * b_pc + b
            for r in range(4):
                xs[32 * r:32 * r + 13, b, :] = S[gb]
                # V for band r: y columns [r*mq + h*ch + j]
                vq = V[gb][:, r * mq:(r + 1) * mq].reshape(13, nh, ch)
                ys[32 * r:32 * r + 13, b, :, r, :] = vq
        in_maps.append({"xs": xs, "ys": ys})
    return in_maps


def finish(results, b_pc=BPC, n=N, m=M):
    """Combine per-core partial outputs into the scalar loss."""
    tot_l = 0.0
    tot_r = 0.0
    nb = 0
    for res in results:
        ml = np.asarray(res["ml_out"], dtype=np.float64)   # [b_pc, nt*nh, 128, ch]
        rm = np.asarray(res["rm_out"], dtype=np.float64)   # [b_pc, 128, 4, mq]
        nt, mq, ch, nh = _dims(n, m)
        # per-chunk [128, ch] partials: min over ch, then over the nh chunks
        mlv = ml.min(axis=3).reshape(b_pc, nt, nh, 128).min(axis=2)
        tot_l += mlv.sum()
        tot_r += rm.min(axis=1).sum()                      # min over partitions
        nb += b_pc
    loss = tot_l / (nb * n) + tot_r / (nb * m)
    return np.float32(loss)


_BUILT = {}


def kernel(x, y):
    x = np.asarray(x)
    y = np.asarray(y)
    assert x.shape == (B, N, D) and y.shape == (B, M, D), (x.shape, y.shape)

    if "nc" not in _BUILT:
        _BUILT["nc"] = build_program()
    nc = _BUILT["nc"]

    in_maps = prep_inputs(x, y)
    core_ids = list(range(NCORES))
    res = run_bass_kernel_spmd(nc, in_maps, core_ids, trace=TRACE)
    LAST["results"] = res
    return finish(res.results)


if __name__ == "__main__":
    xs = np.random.RandomState(0).randn(B, N, D).astype(np.float32)
    ys = np.random.RandomState(1).randn(B, M, D).astype(np.float32)
    print(kernel(xs, ys))
